# revision 15
# baseline (speedup 1.0000x reference)
"""Trainium2 Bass kernel for the DMP (dynamic movement primitives) rollout.

Math: the reference rollout is, per dimension d, a linear 2-state recurrence
    s_t = A s_{t-1} + B u_t,   s = [y; dy],  s_0 = [y0; 0]
with constant A (2x2), B = [dt^2; dt], and forcing
    u_t[d] = ALPHA_Y*BETA_Y*g[d] + sum_j phi_t[j] * weights[d,j]*(g[d]-y0[d])
where phi_t[j] = WEIGHT_SCALE * psi_t[j] * x_t / sum(psi_t) depends only on
constants (x_t = decay^t is input-independent).  By superposition the whole
trajectory factors through an input-independent basis:
    y_t[d], dy_t[d] = sum_m BB[t, comp, m] * coeff[m, d]       (m = 0..26)
with channels m = 0..24 the 25 basis-forced responses (coeff w[:,j]*(g-y0)),
m = 25 the homogeneous response (coeff y0), m = 26 the step response with
ALPHA_Y*BETA_Y folded in (coeff g).

Per core (time rows sharded across 8 cores, no cross-core comm). The kernel
is DMA-write-bound (15.4 MB of output per core vs ~4 us of matmul), so the
structure exists to keep the DMA engines saturated from first dispatch:
  - input loads are spread across the SP and ACT queues so their per-DMA
    sequencer costs overlap,
  - the y0-replica output block is written straight from the y0 DRAM tensor
    with a stride-0 (broadcast) source AP -- no SBUF staging, no
    dependencies, issued on the Pool queue at t~=0; it keeps the DMA
    resource busy while the matmul pipeline ramps,
  - coeff (27 x 1024) is computed on device (DVE stream transposes of w, a
    partition-broadcast multiply by g-y0), f32->f32r via zero-cost bitcast,
  - the y/dy blocks are a [2502, 27] @ [27, 1024] tensor-engine matmul in
    [128, 1024] PSUM tiles; PSUM->SBUF copies alternate DVE/ACT and the
    output writes alternate the SP/Pool queues so no single sequencer lags
    the DMA drain.
"""

import numpy as np

DIM = 1024
NB = 25
ALPHA_X = 1.0
DT = 0.001
MAX_TIME = 10.0
TAU = 1.0
ALPHA_Y = 25.0
BETA_Y = 6.25
WEIGHT_SCALE = 1000.0
T = int(MAX_TIME / DT) + 1        # 10001

NCORES = 8
RPC = 1251                        # t-rows per core; 8*1251 = 10008 >= T
R2 = RPC * 2                      # 2502 matmul rows per core (y and dy)
R2PAD = 2560                      # 20 tiles of 128
NMT = R2PAD // 128                # 20
M = 2 + NB                        # 27 basis channels
# device contraction dim: forced 0..24, zeros 25..31, y0 at 32, g at 33.
# y0/g sit at a quadrant boundary because the engine ops that round them
# into the f32r rhs tile cannot start at partition 25 (quadrant rule);
# the zero basis columns 25..31 contribute nothing to the matmul.
MPAD = 34

_cache = {}


def _basis_slices():
    """Per-core transposed basis slices: list of [M, R2PAD] float32 arrays."""
    if "bbT" in _cache:
        return _cache["bbT"]
    f32 = np.float32
    # phi replicated in fp32 with the reference op order
    c = np.exp(-ALPHA_X * np.linspace(0.0, MAX_TIME, NB, dtype=f32)).astype(f32)
    h = (NB / c).astype(f32)
    decay = f32(1.0 - ALPHA_X * TAU * DT)
    x = f32(1.0)
    phi = np.zeros((T - 1, NB), dtype=np.float64)
    for t in range(T - 1):
        x = f32(x * decay)
        d = (x - c).astype(f32)
        arg = (h * (d * d).astype(f32)).astype(f32)
        psi = np.exp(-arg).astype(f32)
        s = f32(psi.sum(dtype=f32))
        phi[t] = (psi.astype(np.float64) * float(x) * WEIGHT_SCALE) / float(s)

    dt = TAU * DT
    a, b = ALPHA_Y, BETA_Y
    A = np.array([[1 - dt * dt * a * b, dt * (1 - dt * a)],
                  [-dt * a * b, 1 - dt * a]], dtype=np.float64)
    B = np.array([dt * dt, dt], dtype=np.float64)
    # internal channel order: 0 homogeneous (E), 1 step (S), 2.. forced (C)
    Z = np.zeros((2, M), dtype=np.float64)
    Z[0, 0] = 1.0
    # output channel order (must match device rhs rows):
    #   m = 0..24 -> C_j (coeff w.T*(g-y0)); m = 25 -> E (coeff y0);
    #   m = 26 -> ALPHA_Y*BETA_Y*S (coeff g, scale folded into the basis)
    BB = np.zeros((T, 2, M), dtype=np.float64)
    BB[0, 0, 25] = 1.0                 # y_0 = y0 (dy_0 row stays zero)
    u = np.zeros(M)
    u[1] = 1.0
    for t in range(1, T):
        u[2:] = phi[t - 1]
        Z = A @ Z + np.outer(B, u)
        for comp in (0, 1):
            BB[t, comp, :25] = Z[comp, 2:]
            BB[t, comp, 25] = Z[comp, 0]
            BB[t, comp, 26] = (a * b) * Z[comp, 1]

    flat = np.zeros((NCORES * R2, MPAD), dtype=f32)
    fl27 = BB.reshape(T * 2, M).astype(f32)
    flat[: T * 2, 0:NB] = fl27[:, 0:NB]
    flat[: T * 2, 32] = fl27[:, 25]
    flat[: T * 2, 33] = fl27[:, 26]
    slices = []
    for i in range(NCORES):
        bbT = np.zeros((MPAD, R2PAD), dtype=f32)
        bbT[:, :R2] = flat[i * R2:(i + 1) * R2].T
        slices.append(np.ascontiguousarray(bbT))
    _cache["bbT"] = slices
    return slices


def _program():
    """Build (once) the Bass/Tile program shared by all 8 cores."""
    if "nc" in _cache:
        return _cache["nc"]
    import concourse.mybir as mybir
    import concourse.tile as tile
    from concourse import bacc

    f32 = mybir.dt.float32
    f32r = mybir.dt.float32r
    nc = bacc.Bacc("TRN2", target_bir_lowering=False, debug=False,
                   enable_asserts=False, num_devices=NCORES)
    bbT_h = nc.dram_tensor("bbT", [MPAD, R2PAD], f32, kind="ExternalInput")
    y0_h = nc.dram_tensor("y0", [1, DIM], f32, kind="ExternalInput")
    g_h = nc.dram_tensor("g", [1, DIM], f32, kind="ExternalInput")
    w_h = nc.dram_tensor("w", [8, 128, NB], f32, kind="ExternalInput")
    out_h = nc.dram_tensor("out", [RPC, 3, DIM], f32, kind="ExternalOutput")

    with tile.TileContext(nc) as tc:
        with (
            tc.tile_pool(name="const", bufs=1) as const,
            tc.tile_pool(name="psMM", bufs=3, space="PSUM") as psMM,
            tc.tile_pool(name="psAux", bufs=1, space="PSUM") as psAux,
            tc.tile_pool(name="outp", bufs=6) as outp,
        ):
            outv = out_h.ap()

            # ---- input loads, spread across SP and ACT queues ----
            y0_s = const.tile([1, DIM], f32)
            nc.sync.dma_start(y0_s[:], y0_h.ap()[:])
            g_s = const.tile([1, DIM], f32)
            nc.sync.dma_start(g_s[:], g_h.ap()[:])
            bb_s = const.tile([MPAD, R2PAD], f32)
            nc.scalar.dma_start(bb_s[:], bbT_h.ap()[:])
            # weights tiles, free dim padded 25 -> 32 per block for the 32x32
            # DVE stream transposes (padding cols stay uninitialized: they
            # only transpose into wt rows 25..31, which are never read)
            w_s = const.tile([128, 8 * 32], f32)
            nc.scalar.dma_start(
                w_s[:].rearrange("p (a j) -> p a j", a=8)[:, :, 0:NB],
                w_h.ap().rearrange("a p j -> p a j"))
            # y0/g stacked on two adjacent partitions: source for the single
            # rounding copy into rhs rows 32/33 (DMA APs need no quadrant
            # alignment, the engine copy does)
            yg_s = const.tile([2, DIM], f32)
            nc.scalar.dma_start(yg_s[0:1, :], y0_h.ap()[:])
            nc.scalar.dma_start(yg_s[1:2, :], g_h.ap()[:])
            # rhs is f32r: every producer is an engine op that rounds on
            # write (the BIR verifier rejects DMA- or bitcast-produced
            # f32r matmul operands)
            rhs_s = const.tile([MPAD, DIM], f32r)
            # zero the pad rows 25..31 (quadrant rule: start at partition 0,
            # the mul below overwrites rows 0..24; memset cannot emit f32r,
            # so bounce zeros through an f32 tile and a rounding copy)
            z_s = const.tile([32, DIM], f32)
            nc.vector.memset(z_s[:], 0.0)
            nc.vector.tensor_copy(rhs_s[0:32, :], z_s[:])

            # ---- y0-replica output block ----
            # Written straight from the y0 DRAM tensor with a stride-0
            # broadcast source: no SBUF staging, no dependencies.  Two
            # chunks so the SWDGE descriptor ring is not overrun and input
            # loads can slot in between.
            H1 = 640
            nc.gpsimd.dma_start(outv[0:H1, 0, :],
                                y0_h.ap().broadcast_to([H1, DIM]))
            nc.gpsimd.dma_start(outv[H1:RPC, 0, :],
                                y0_h.ap().broadcast_to([RPC - H1, DIM]))

            # ---- coeff (rhs) assembly ----
            gmy0 = const.tile([1, DIM], f32)
            nc.vector.tensor_sub(gmy0[:], g_s[:], y0_s[:])
            # partition-broadcast g-y0 to 25 rows via a PE outer product
            # (ones [1,25]^T @ gmy0 [1,1024]); engine APs reject stride-0
            # partitions and a DMA round-trip would queue behind the bulk
            # y0-block writes on the shared DMA resource
            ones_s = const.tile([1, NB], f32)
            nc.vector.memset(ones_s[:], 1.0)
            rep_ps = psAux.tile([NB, DIM], f32)
            nc.tensor.matmul(rep_ps[:, 0:512], ones_s[:], gmy0[:, 0:512],
                             start=True, stop=True)
            nc.tensor.matmul(rep_ps[:, 512:1024], ones_s[:], gmy0[:, 512:1024],
                             start=True, stop=True)

            # w.T via DVE 32x32 stream transposes
            wt_s = const.tile([32, 8 * 128], f32)
            for a in range(8):
                for i in range(4):
                    nc.vector.transpose(
                        wt_s[:, a * 128 + 32 * i:a * 128 + 32 * (i + 1)],
                        w_s[32 * i:32 * (i + 1), a * 32:(a + 1) * 32])

            # rows 0..24: w.T * (g - y0)  (second operand read from PSUM);
            # rows 32/33: y0, g.  Both producers write the f32r tile, so
            # the engines round on write.
            nc.vector.tensor_mul(rhs_s[0:NB, :], wt_s[0:NB, :], rep_ps[:])
            nc.scalar.copy(rhs_s[32:34, :], yg_s[:])

            # lhsT must also be engine-rounded to f32r (the verifier rejects
            # a bitcast view of the DMA-loaded f32 data)
            bb2 = const.tile([MPAD, R2PAD], f32r)
            nc.vector.tensor_copy(bb2[:], bb_s[:])
            rhs2 = rhs_s[:]

            # ---- main matmul: [2502, 27] @ [27, 1024] in [128, 1024] PSUM
            # tiles; each 128-row tile covers 64 t-rows x {y, dy} ----
            for mt in range(NMT):
                ms = slice(mt * 128, (mt + 1) * 128)
                ps = psMM.tile([128, DIM], f32)
                nc.tensor.matmul(ps[:, 0:512], bb2[:, ms], rhs2[:, 0:512],
                                 start=True, stop=True)
                nc.tensor.matmul(ps[:, 512:1024], bb2[:, ms], rhs2[:, 512:1024],
                                 start=True, stop=True)
                ob = outp.tile([128, DIM], f32)
                if mt % 2 == 0:
                    nc.vector.tensor_copy(ob[:], ps[:])
                else:
                    nc.scalar.copy(ob[:], ps[:])
                t0 = mt * 64
                tv = min(64, RPC - t0)
                eng = nc.sync if mt % 2 == 0 else nc.gpsimd
                eng.dma_start(outv[t0:t0 + tv, 1:3, :], ob[:2 * tv, :])

    nc.compile()   # bacc passes: wait legalization (1-wait HW cap), regalloc
    _cache["nc"] = nc
    return nc


def _run(in_maps, **kwargs):
    from concourse.bass_utils import run_bass_kernel_spmd
    return run_bass_kernel_spmd(_program(), in_maps, core_ids=list(range(NCORES)),
                                **kwargs)


def _in_maps(y0, g, weights):
    f32 = np.float32
    y0f = np.ascontiguousarray(np.asarray(y0, f32).reshape(1, DIM))
    gf = np.ascontiguousarray(np.asarray(g, f32).reshape(1, DIM))
    wf = np.ascontiguousarray(np.asarray(weights, f32).reshape(8, 128, NB))
    return [{"bbT": bbT, "y0": y0f, "g": gf, "w": wf}
            for bbT in _basis_slices()]


def kernel(y0, g, weights, **_kwargs):
    res = _run(_in_maps(y0, g, weights))
    outs = [r["out"].reshape(RPC, 3 * DIM) for r in res.results]
    return np.ascontiguousarray(np.concatenate(outs, axis=0)[:T])


# revision 19
# speedup vs baseline: 1.1420x; 1.1420x over previous
"""Trainium2 Bass kernel for the DMP (dynamic movement primitives) rollout.

Math: the reference rollout is, per dimension d, a linear 2-state recurrence
    s_t = A s_{t-1} + B u_t,   s = [y; dy],  s_0 = [y0; 0]
with constant A (2x2), B = [dt^2; dt], and forcing
    u_t[d] = ALPHA_Y*BETA_Y*g[d] + sum_j phi_t[j] * weights[d,j]*(g[d]-y0[d])
where phi_t[j] = WEIGHT_SCALE * psi_t[j] * x_t / sum(psi_t) depends only on
constants (x_t = decay^t is input-independent).  By superposition the whole
trajectory factors through an input-independent basis:
    y_t[d], dy_t[d] = sum_m BB[t, comp, m] * coeff[m, d]       (m = 0..26)
with channels m = 0..24 the 25 basis-forced responses (coeff w[:,j]*(g-y0)),
m = 25 the homogeneous response (coeff y0), m = 26 the step response with
ALPHA_Y*BETA_Y folded in (coeff g).

Per core (time rows sharded across 8 cores, no cross-core comm). The kernel
is DMA-write-bound (15.4 MB of output per core vs ~4 us of matmul), so the
structure exists to keep the DMA engines saturated from first dispatch:
  - input loads are spread across the SP and ACT queues so their per-DMA
    sequencer costs overlap,
  - the y0-replica output block is written straight from the y0 DRAM tensor
    with a stride-0 (broadcast) source AP -- no SBUF staging, no
    dependencies, issued on the Pool queue at t~=0; it keeps the DMA
    resource busy while the matmul pipeline ramps,
  - coeff (27 x 1024) is computed on device (DVE stream transposes of w, a
    partition-broadcast multiply by g-y0), f32->f32r via zero-cost bitcast,
  - the y/dy blocks are a [2502, 27] @ [27, 1024] tensor-engine matmul in
    [128, 1024] PSUM tiles; PSUM->SBUF copies alternate DVE/ACT and the
    output writes alternate the SP/Pool queues so no single sequencer lags
    the DMA drain.
"""

import numpy as np

DIM = 1024
NB = 25
ALPHA_X = 1.0
DT = 0.001
MAX_TIME = 10.0
TAU = 1.0
ALPHA_Y = 25.0
BETA_Y = 6.25
WEIGHT_SCALE = 1000.0
T = int(MAX_TIME / DT) + 1        # 10001

NCORES = 8
RPC = 1251                        # t-rows per core; 8*1251 = 10008 >= T
R2 = RPC * 2                      # 2502 matmul rows per core (y and dy)
R2PAD = 2560                      # 20 tiles of 128
NMT = R2PAD // 128                # 20
M = 2 + NB                        # 27 basis channels
# device contraction dim: forced 0..24, zeros 25..31, y0 at 32, g at 33.
# y0/g sit at a quadrant boundary because the engine ops that round them
# into the f32r rhs tile cannot start at partition 25 (quadrant rule);
# the zero basis columns 25..31 contribute nothing to the matmul.
MPAD = 34

_cache = {}


def _basis_slices():
    """Per-core transposed basis slices: list of [M, R2PAD] float32 arrays."""
    if "bbT" in _cache:
        return _cache["bbT"]
    f32 = np.float32
    # phi replicated in fp32 with the reference op order
    c = np.exp(-ALPHA_X * np.linspace(0.0, MAX_TIME, NB, dtype=f32)).astype(f32)
    h = (NB / c).astype(f32)
    decay = f32(1.0 - ALPHA_X * TAU * DT)
    x = f32(1.0)
    phi = np.zeros((T - 1, NB), dtype=np.float64)
    for t in range(T - 1):
        x = f32(x * decay)
        d = (x - c).astype(f32)
        arg = (h * (d * d).astype(f32)).astype(f32)
        psi = np.exp(-arg).astype(f32)
        s = f32(psi.sum(dtype=f32))
        phi[t] = (psi.astype(np.float64) * float(x) * WEIGHT_SCALE) / float(s)

    dt = TAU * DT
    a, b = ALPHA_Y, BETA_Y
    A = np.array([[1 - dt * dt * a * b, dt * (1 - dt * a)],
                  [-dt * a * b, 1 - dt * a]], dtype=np.float64)
    B = np.array([dt * dt, dt], dtype=np.float64)
    # internal channel order: 0 homogeneous (E), 1 step (S), 2.. forced (C)
    Z = np.zeros((2, M), dtype=np.float64)
    Z[0, 0] = 1.0
    # output channel order (must match device rhs rows):
    #   m = 0..24 -> C_j (coeff w.T*(g-y0)); m = 25 -> E (coeff y0);
    #   m = 26 -> ALPHA_Y*BETA_Y*S (coeff g, scale folded into the basis)
    BB = np.zeros((T, 2, M), dtype=np.float64)
    BB[0, 0, 25] = 1.0                 # y_0 = y0 (dy_0 row stays zero)
    u = np.zeros(M)
    u[1] = 1.0
    for t in range(1, T):
        u[2:] = phi[t - 1]
        Z = A @ Z + np.outer(B, u)
        for comp in (0, 1):
            BB[t, comp, :25] = Z[comp, 2:]
            BB[t, comp, 25] = Z[comp, 0]
            BB[t, comp, 26] = (a * b) * Z[comp, 1]

    flat = np.zeros((NCORES * R2, MPAD), dtype=f32)
    fl27 = BB.reshape(T * 2, M).astype(f32)
    flat[: T * 2, 0:NB] = fl27[:, 0:NB]
    flat[: T * 2, 32] = fl27[:, 25]
    flat[: T * 2, 33] = fl27[:, 26]
    slices = []
    for i in range(NCORES):
        bbT = np.zeros((MPAD, R2PAD), dtype=f32)
        bbT[:, :R2] = flat[i * R2:(i + 1) * R2].T
        slices.append(np.ascontiguousarray(bbT))
    _cache["bbT"] = slices
    return slices


def _program():
    """Build (once) the Bass/Tile program shared by all 8 cores."""
    if "nc" in _cache:
        return _cache["nc"]
    import concourse.mybir as mybir
    import concourse.tile as tile
    from concourse import bacc

    f32 = mybir.dt.float32
    f32r = mybir.dt.float32r
    nc = bacc.Bacc("TRN2", target_bir_lowering=False, debug=False,
                   enable_asserts=False, num_devices=NCORES)
    bbT_h = nc.dram_tensor("bbT", [MPAD, R2PAD], f32, kind="ExternalInput")
    y0_h = nc.dram_tensor("y0", [1, DIM], f32, kind="ExternalInput")
    g_h = nc.dram_tensor("g", [1, DIM], f32, kind="ExternalInput")
    w_h = nc.dram_tensor("w", [8, 128, NB], f32, kind="ExternalInput")
    out_h = nc.dram_tensor("out", [RPC, 3, DIM], f32, kind="ExternalOutput")

    with tile.TileContext(nc) as tc:
        with (
            tc.tile_pool(name="const", bufs=1) as const,
            tc.tile_pool(name="psMM", bufs=3, space="PSUM") as psMM,
            tc.tile_pool(name="psAux", bufs=1, space="PSUM") as psAux,
            tc.tile_pool(name="outp", bufs=6) as outp,
        ):
            outv = out_h.ap()

            # ---- input loads, spread across SP and ACT queues ----
            y0_s = const.tile([1, DIM], f32)
            nc.sync.dma_start(y0_s[:], y0_h.ap()[:])
            g_s = const.tile([1, DIM], f32)
            nc.sync.dma_start(g_s[:], g_h.ap()[:])
            # basis rows 25..31 are zero: don't move them over DMA, the
            # zeros are produced on-engine into bb2 below
            bb_s = const.tile([MPAD, R2PAD], f32)
            nc.scalar.dma_start(bb_s[0:NB, :], bbT_h.ap()[0:NB, :])
            nc.scalar.dma_start(bb_s[32:34, :], bbT_h.ap()[32:34, :])
            # weights tiles, free dim padded 25 -> 32 per block for the 32x32
            # DVE stream transposes (padding cols stay uninitialized: they
            # only transpose into wt rows 25..31, which are never read)
            w_s = const.tile([128, 8 * 32], f32)
            nc.scalar.dma_start(
                w_s[:].rearrange("p (a j) -> p a j", a=8)[:, :, 0:NB],
                w_h.ap().rearrange("a p j -> p a j"))
            # y0/g stacked on two adjacent partitions: source for the single
            # rounding copy into rhs rows 32/33 (DMA APs need no quadrant
            # alignment, the engine copy does)
            yg_s = const.tile([2, DIM], f32)
            nc.scalar.dma_start(yg_s[0:1, :], y0_h.ap()[:])
            nc.scalar.dma_start(yg_s[1:2, :], g_h.ap()[:])
            # rhs is f32r: every producer is an engine op that rounds on
            # write (the BIR verifier rejects DMA- or bitcast-produced
            # f32r matmul operands)
            rhs_s = const.tile([MPAD, DIM], f32r)
            # zero the pad rows 25..31 of both f32r operands (quadrant rule:
            # start at partition 0, the real rows are overwritten after;
            # memset cannot emit f32r, so bounce zeros through an f32 tile
            # and rounding copies)
            z_s = const.tile([32, R2PAD], f32)
            nc.vector.memset(z_s[:], 0.0)
            nc.vector.tensor_copy(rhs_s[0:32, :], z_s[:, 0:DIM])

            # ---- y0-replica output block ----
            # Written straight from the y0 DRAM tensor with a stride-0
            # broadcast source: no SBUF staging, no dependencies.  Chunked
            # ~2.4 us apiece: the DMA engines are modeled (and behave) as
            # one exclusive resource, so waiting input loads slot in at
            # chunk boundaries instead of starving behind one 14 us write.
            NCH = 6
            csz = (RPC + NCH - 1) // NCH
            for c0 in range(0, RPC, csz):
                cn = min(csz, RPC - c0)
                nc.gpsimd.dma_start(outv[c0:c0 + cn, 0, :],
                                    y0_h.ap().broadcast_to([cn, DIM]))

            # ---- coeff (rhs) assembly ----
            gmy0 = const.tile([1, DIM], f32)
            nc.vector.tensor_sub(gmy0[:], g_s[:], y0_s[:])
            # partition-broadcast g-y0 to 25 rows via a PE outer product
            # (ones [1,25]^T @ gmy0 [1,1024]); engine APs reject stride-0
            # partitions and a DMA round-trip would queue behind the bulk
            # y0-block writes on the shared DMA resource
            ones_s = const.tile([1, NB], f32)
            nc.vector.memset(ones_s[:], 1.0)
            rep_ps = psAux.tile([NB, DIM], f32)
            nc.tensor.matmul(rep_ps[:, 0:512], ones_s[:], gmy0[:, 0:512],
                             start=True, stop=True)
            nc.tensor.matmul(rep_ps[:, 512:1024], ones_s[:], gmy0[:, 512:1024],
                             start=True, stop=True)

            # w.T via DVE 32x32 stream transposes
            wt_s = const.tile([32, 8 * 128], f32)
            for a in range(8):
                for i in range(4):
                    nc.vector.transpose(
                        wt_s[:, a * 128 + 32 * i:a * 128 + 32 * (i + 1)],
                        w_s[32 * i:32 * (i + 1), a * 32:(a + 1) * 32])

            # rows 0..24: w.T * (g - y0)  (second operand read from PSUM);
            # rows 32/33: y0, g.  Both producers write the f32r tile, so
            # the engines round on write.
            nc.vector.tensor_mul(rhs_s[0:NB, :], wt_s[0:NB, :], rep_ps[:])
            nc.scalar.copy(rhs_s[32:34, :], yg_s[:])

            # lhsT must also be engine-rounded to f32r (the verifier rejects
            # a bitcast view of the DMA-loaded f32 data); rows 25..31 come
            # from the zero tile, rows 0..24 and 32..33 from the load
            bb2 = const.tile([MPAD, R2PAD], f32r)
            nc.vector.tensor_copy(bb2[0:32, :], z_s[:])
            nc.vector.tensor_copy(bb2[0:NB, :], bb_s[0:NB, :])
            nc.vector.tensor_copy(bb2[32:34, :], bb_s[32:34, :])
            rhs2 = rhs_s[:]

            # ---- main matmul: [2502, 27] @ [27, 1024] in [128, 1024] PSUM
            # tiles; each 128-row tile covers 64 t-rows x {y, dy} ----
            for mt in range(NMT):
                ms = slice(mt * 128, (mt + 1) * 128)
                ps = psMM.tile([128, DIM], f32)
                nc.tensor.matmul(ps[:, 0:512], bb2[:, ms], rhs2[:, 0:512],
                                 start=True, stop=True)
                nc.tensor.matmul(ps[:, 512:1024], bb2[:, ms], rhs2[:, 512:1024],
                                 start=True, stop=True)
                ob = outp.tile([128, DIM], f32)
                if mt % 2 == 0:
                    nc.vector.tensor_copy(ob[:], ps[:])
                else:
                    nc.scalar.copy(ob[:], ps[:])
                t0 = mt * 64
                tv = min(64, RPC - t0)
                eng = nc.sync if mt % 2 == 0 else nc.gpsimd
                eng.dma_start(outv[t0:t0 + tv, 1:3, :], ob[:2 * tv, :])

    nc.compile()   # bacc passes: wait legalization (1-wait HW cap), regalloc
    _cache["nc"] = nc
    return nc


def _run(in_maps, **kwargs):
    from concourse.bass_utils import run_bass_kernel_spmd
    return run_bass_kernel_spmd(_program(), in_maps, core_ids=list(range(NCORES)),
                                **kwargs)


def _in_maps(y0, g, weights):
    f32 = np.float32
    y0f = np.ascontiguousarray(np.asarray(y0, f32).reshape(1, DIM))
    gf = np.ascontiguousarray(np.asarray(g, f32).reshape(1, DIM))
    wf = np.ascontiguousarray(np.asarray(weights, f32).reshape(8, 128, NB))
    return [{"bbT": bbT, "y0": y0f, "g": gf, "w": wf}
            for bbT in _basis_slices()]


def kernel(y0, g, weights, **_kwargs):
    res = _run(_in_maps(y0, g, weights))
    outs = [r["out"].reshape(RPC, 3 * DIM) for r in res.results]
    return np.ascontiguousarray(np.concatenate(outs, axis=0)[:T])
